# revision 36
# baseline (speedup 1.0000x reference)
"""TRN2 Bass kernel for nn_AttentionWrapper (GQA attention + RoPE + causal mask
+ post-softmax suppression), tensor-parallel over heads across 8 NeuronCores.

Sharding: core i owns q-heads 4i..4i+3 and kv-head i (GQA group i), i.e. rows
512i..512(i+1) of Wq, rows 128i..128(i+1) of Wk/Wv, and columns 512i..512(i+1)
of Wo. hidden_states is replicated; each core emits a full-shape partial of
the output projection (bf16) and the host sums the 8 partials in fp32.

V8 (vs V3): P1's last K tile runs ob-major so each PSUM accumulator finishes
4 matmuls apart and its eviction + RoPE chain (rot matmul deferred one
ob-group) overlaps the next accumulator's matmuls — no eviction burst at
chunk boundaries and ScalarE is already free when P2's first exps arrive.
P2's binding resource is ScalarE (exp ~690ns/block vs PE 216ns
score matmuls, and scpool's 3-bank rotation throttles the score stream to exp
rate). The previous query-chunk's P3 units (4 Wo matmuls + eviction each) are
interleaved one-per-two score blocks as PE filler, with demand-matched
flushing (pending drains to what the next chunk's slots can absorb), so the
PE never idles waiting on exp and HAM stays warm. The GpSimd
partition_all_reduce (3.5us on the normalization critical path) is replaced
by a one-instruction PE reduce: strips evict to bf16 SBUF, a ones[128,128]
matmul re-sums them into a recycled PSUM bank, reciprocal reads PSUM. Output
partials are written bf16 (halves out DMA), aggregated into one [128, 4096]
SBUF tile per token block so each output DMA is one descriptor set of 8KB
lines (the per-(tb,mc) 1KB-line DMAs were descriptor-rate bound and trailed
the PE by ~8us at the end). Startup: the first X subtiles ride the scalar DMA
queue right behind the first weight block (sync-queue spin-up lags by ~7us);
constants follow the weights on the scalar queue; P1 PSUM evictions alternate
Scalar/Vector so ScalarE is free for P2's first exps.

All intermediates (QT/KT/V/OT) live in SBUF for the whole kernel; all matmul
operands are bf16 (fp32 PSUM accumulation); V is transposed with the DMA XBAR;
normalization uses reciprocal_approx_fast.
"""
import math
from collections import deque

import ml_dtypes
import numpy as np

import concourse.bass as bass
import concourse.bacc as bacc
import concourse.bass_isa as bass_isa
import concourse.mybir as mybir
import concourse.tile as tile
from concourse.bass_utils import run_bass_kernel_spmd

B, S, HID = 2, 2048, 4096
NH, NKV, HD = 32, 8, 128
NCORES = 8
HQ = NH // NCORES            # 4 q heads per core
T = B * S                    # 4096 token axis (b*S + s)
THETA = 10000.0
SUPPRESS = 0.1
SCALE = 1.0 / math.sqrt(HD)

P = 128
TC = 512                     # P1 token chunk
NCH = T // TC                # 8
KS = 4                       # k subtiles per K tile (K_TILE = 512)
NKT = HID // (P * KS)        # 8 K tiles over HID
WC = 512 + HD + HD           # 768 fused wqkv columns (4 q heads, k, v)
QC = 512                     # P2 query chunk
NQC = S // QC                # 4 per batch
NKB = S // P                 # 16 key blocks per batch
MC = 512                     # P3 output column chunk
BF = mybir.dt.bfloat16
F32 = mybir.dt.float32

QC_ORDER = {0: (0, 1, 2, 3), 1: (3, 2, 1, 0)}


def _build_program(sup_plan, causal):
    """sup_plan: list over b of (qc_idx, tloc, sorted kb list).
    causal: per-batch bool - True applies the causal structure (variable-width
    score matmuls + triangular diag mask), False runs full attention."""
    nc = bacc.Bacc("TRN2", target_bir_lowering=False, debug=False)

    xt = nc.dram_tensor("xt", [NCH, NKT, P, KS, TC], BF, kind="ExternalInput")
    wqkv = nc.dram_tensor("wqkv", [NKT, P, KS, WC], BF, kind="ExternalInput")
    wo = nc.dram_tensor("wo", [HQ * HD, HID], BF, kind="ExternalInput")
    cs = nc.dram_tensor("cs", [HD, T], BF, kind="ExternalInput")
    sn = nc.dram_tensor("sn", [HD, T], BF, kind="ExternalInput")
    rmat = nc.dram_tensor("rmat", [HD, HD], BF, kind="ExternalInput")
    tri = nc.dram_tensor("tri", [P, P], BF, kind="ExternalInput")
    ones = nc.dram_tensor("ones", [P, P], BF, kind="ExternalInput")
    on32 = nc.dram_tensor("on32", [P, 32], BF, kind="ExternalInput")
    sup = nc.dram_tensor("sup", [P, B, NKB], F32, kind="ExternalInput")
    out = nc.dram_tensor("out", [T, HID], BF, kind="ExternalOutput")

    wo3 = wo.rearrange("(jb p) m -> p jb m", p=P)

    with tile.TileContext(nc) as tc:
        with (
            tc.tile_pool(name="const", bufs=1) as cpool,
            tc.tile_pool(name="persist", bufs=1) as perpool,
            tc.tile_pool(name="p3w", bufs=1) as w3pool,
        ):
            # Constants ride the scalar DMA queue behind the weight stream
            # (they are small and first needed at the ch0 epilogue / P2);
            # the sync queue opens with chunk X tiles only.
            rmat_sb = cpool.tile([HD, HD], BF, tag="rmat")
            tri_sb = cpool.tile([P, P], BF, tag="tri")
            ones_sb = cpool.tile([P, P], BF, tag="ones")
            on32_sb = cpool.tile([P, 32], BF, tag="on32")
            sup_sb = cpool.tile([P, B, NKB], F32, tag="sup")

            # Persistent SBUF intermediates (bf16, per-partition bytes):
            # QT 32K, KT 8K, V 8K, OT 32K.
            qt_all = perpool.tile([P, B, HQ, S], BF, tag="qt")
            kt_all = perpool.tile([P, B, S], BF, tag="kt")
            v_all = perpool.tile([P, B, NKB, HD], BF, tag="v")
            ot_all = perpool.tile([P, B, HQ, S], BF, tag="ot")

            wo_sb = w3pool.tile([P, HQ, HID], BF, tag="wo")

            # ---- P1: projections + RoPE ---------------------------------
            with (
                tc.tile_pool(name="p1w", bufs=1) as wpool,
                tc.tile_pool(name="p1x", bufs=3) as xpool,
                tc.tile_pool(name="p1cs", bufs=2) as cspool,
                tc.tile_pool(name="p1raw", bufs=3) as rawpool,
                tc.tile_pool(name="p1vt", bufs=2) as vtpool,
                tc.tile_pool(name="p1tmp", bufs=4) as tmppool,
                tc.tile_pool(name="p1ps", bufs=6, space="PSUM") as pspool,
                tc.tile_pool(name="p1rot", bufs=2, space="PSUM") as rotpool,
            ):
                # Weight stream on the scalar DMA queue. Dedicated tile for
                # the very first k-subtile of weights so the first matmuls
                # wait only on its own small DMA, not the whole first weight
                # tile. The first X tile's subtiles ride the same scalar
                # queue right behind it (the sync queue has its own multi-us
                # spin-up; this way the PE's first matmul waits on one warm
                # queue only).
                wt0a = wpool.tile([P, WC], BF, tag="w0a", name="wt0a")
                nc.scalar.dma_start(wt0a[:], wqkv[0, :, 0])
                x0tile = xpool.tile([P, KS, TC], BF, tag="x", name="x0")
                for ks_ in range(KS):
                    nc.scalar.dma_start(x0tile[:, ks_], xt[0, 0, :, ks_])
                wts = []
                for kt in range(NKT):
                    wt = wpool.tile([P, KS, WC], BF, tag=f"w{kt}",
                                    name=f"wt{kt}")
                    if kt == 0:
                        for ks_ in range(1, KS):
                            nc.scalar.dma_start(wt[:, ks_], wqkv[kt, :, ks_])
                    else:
                        nc.scalar.dma_start(wt[:], wqkv[kt])
                    wts.append(wt)
                nc.scalar.dma_start(rmat_sb[:], rmat[:])
                nc.scalar.dma_start(tri_sb[:], tri[:])
                nc.scalar.dma_start(ones_sb[:], ones[:])
                nc.scalar.dma_start(on32_sb[:], on32[:])
                nc.scalar.dma_start(sup_sb[:], sup[:])

                for ch in range(NCH):
                    b = ch // (NCH // B)
                    if ch == 1:
                        # Wo prefetch once the chunk-0 V transposes are
                        # queued; needed only at P2/P3 time.
                        nc.scalar.dma_start(wo_sb[:], wo3[:])
                    xtiles = []
                    for kt in range(NKT):
                        if ch == 0 and kt == 0:
                            xtiles.append(x0tile)
                            continue
                        xtile = xpool.tile([P, KS, TC], BF, tag="x")
                        nc.sync.dma_start(xtile[:], xt[ch, kt])
                        xtiles.append(xtile)
                    cs_t = cspool.tile([HD, TC], BF, tag="cs")
                    nc.sync.dma_start(cs_t[:], cs[:, ch * TC:(ch + 1) * TC])
                    sn_t = cspool.tile([HD, TC], BF, tag="sn")
                    nc.sync.dma_start(sn_t[:], sn[:, ch * TC:(ch + 1) * TC])

                    pss = [pspool.tile([P, TC], F32, tag="acc",
                                       name=f"acc{ch}_{i}")
                           for i in range(6)]
                    for kt in range(NKT - 1):
                        for ks_ in range(KS):
                            first = (kt == 0 and ks_ == 0)
                            for ob in range(6):
                                w_ap = (wt0a[:, ob * P:(ob + 1) * P] if first
                                        else wts[kt][:, ks_,
                                                     ob * P:(ob + 1) * P])
                                nc.tensor.matmul(
                                    pss[ob], w_ap, xtiles[kt][:, ks_],
                                    start=(kt == 0 and ks_ == 0),
                                    stop=False)

                    def evict_ob(ob):
                        # PSUM eviction only (off-PE); returns the raw tile
                        # for the deferred RoPE chain. Alternate engines so
                        # neither ScalarE nor DVE serializes the epilogue.
                        ps = pss[ob]
                        if ob < 5:
                            raw = rawpool.tile([P, TC], BF, tag="raw",
                                               name=f"raw{ch}_{ob}")
                            if ob % 2 == 0:
                                nc.scalar.copy(raw[:], ps[:])
                            else:
                                nc.vector.tensor_copy(raw[:], ps[:])
                            return raw
                        vt = vtpool.tile([P, TC], BF, tag="vt")
                        nc.scalar.copy(vt[:], ps[:])
                        for tb in range(TC // P):
                            gkb = (ch % (NCH // B)) * (TC // P) + tb
                            nc.sync.dma_start(
                                v_all[:, b, gkb, :],
                                vt[:, tb * P:(tb + 1) * P],
                                transpose=True)
                        return None

                    def rope_ob(ob, raw):
                        rot = rotpool.tile([P, TC], F32)
                        nc.tensor.matmul(rot[:], rmat_sb[:], raw[:],
                                         start=True, stop=True)
                        t1 = tmppool.tile([P, TC], BF, tag="t1")
                        nc.vector.tensor_mul(t1[:], raw[:], cs_t[:])
                        t2 = tmppool.tile([P, TC], BF, tag="t2")
                        nc.vector.tensor_mul(t2[:], rot[:], sn_t[:])
                        soff = (ch % (NCH // B)) * TC
                        if ob < HQ:
                            dest = qt_all[:, b, ob, soff:soff + TC]
                        else:
                            dest = kt_all[:, b, soff:soff + TC]
                        if ch % (NCH // B) == NCH // B - 1:
                            # last chunk of this batch: do the final add
                            # on GpSimd so P2's first score matmuls don't
                            # chain on the whole DVE completion counter.
                            nc.gpsimd.tensor_add(dest, t1[:], t2[:])
                        else:
                            nc.vector.tensor_add(dest, t1[:], t2[:])

                    # Last K tile runs ob-major: each accumulator takes its
                    # final 4 matmuls in turn, so its eviction overlaps the
                    # next accumulator's matmuls instead of bursting at the
                    # chunk boundary (which stalled the next chunk's first
                    # matmuls on freed banks and kept ScalarE busy exactly
                    # when P2 needs exps). Each RoPE rot matmul is deferred
                    # one ob-group so it never waits on its eviction copy.
                    raws = {}
                    for ob in range(6):
                        for ks_ in range(KS):
                            nc.tensor.matmul(
                                pss[ob],
                                wts[NKT - 1][:, ks_, ob * P:(ob + 1) * P],
                                xtiles[NKT - 1][:, ks_],
                                start=False, stop=(ks_ == KS - 1))
                        raws[ob] = evict_ob(ob)
                        if ob >= 1 and raws.get(ob - 1) is not None:
                            rope_ob(ob - 1, raws.pop(ob - 1))

            # ---- P2 + P3 interleaved ------------------------------------
            # pending: P3 units of the previous query chunk, drained as PE
            # filler one-per-two score blocks while ScalarE streams exps.
            with (
                tc.tile_pool(name="p2pt", bufs=3) as ptpool,
                tc.tile_pool(name="p2sme", bufs=2) as smepool,
                tc.tile_pool(name="p2rec", bufs=2) as recpool,
                tc.tile_pool(name="p3e", bufs=3) as e3pool,
                tc.tile_pool(name="p2sc", bufs=3, space="PSUM") as scpool,
                tc.tile_pool(name="p2sm", bufs=1, space="PSUM") as smpool,
                tc.tile_pool(name="p2op", bufs=2, space="PSUM") as oppool,
                tc.tile_pool(name="p3ps", bufs=2, space="PSUM") as ps3pool,
            ):
                pending = deque()

                def nkb_of(b, qc):
                    return (qc + 1) * (QC // P) if causal[b] else NKB

                def slots_of(b, qc):
                    # fill opportunities inside a (b, qc) iteration: one per
                    # two score blocks per head, one per sums_ot_norm, plus
                    # the trailing fill(2).
                    return HQ * (nkb_of(b, qc) // 2) + HQ + 2

                order = [(b, qc) for b in range(B) for qc in QC_ORDER[b]]

                ev_state = {}          # tb key -> ev tile

                def p3_unit(bp, tb, mc):
                    ps = ps3pool.tile([P, MC], F32)
                    for jb in range(HQ):
                        nc.tensor.matmul(
                            ps[:],
                            ot_all[:, bp, jb, tb * P:(tb + 1) * P],
                            wo_sb[:, jb, mc * MC:(mc + 1) * MC],
                            start=(jb == 0), stop=(jb == HQ - 1))
                    # aggregate the 8 column chunks of a token block into one
                    # [P, HID] tile and ship a single 1MB DMA per tb: one
                    # descriptor set of 8KB lines instead of eight of 1KB,
                    # alternating HWDGE queues so the write stream never
                    # falls behind the PE at the end of the kernel.
                    key = (bp, tb)
                    if key not in ev_state:
                        ev_state[key] = e3pool.tile([P, HID], BF, tag="ev",
                                                    name=f"ev{bp}_{tb}")
                    ev = ev_state[key]
                    nc.vector.tensor_copy(ev[:, mc * MC:(mc + 1) * MC], ps[:])
                    if mc == HID // (2 * MC) - 1:
                        # ship the first half as soon as it completes so the
                        # write overlaps the second half's evictions.
                        nc.sync.dma_start(
                            out[bp * S + tb * P:bp * S + (tb + 1) * P,
                                0:HID // 2],
                            ev[:, 0:HID // 2])
                    elif mc == HID // MC - 1:
                        del ev_state[key]
                        nc.sync.dma_start(
                            out[bp * S + tb * P:bp * S + (tb + 1) * P,
                                HID // 2:],
                            ev[:, HID // 2:])

                def fill(n):
                    for _ in range(min(n, len(pending))):
                        p3_unit(*pending.popleft())

                for oi, (b, qc) in enumerate(order):
                    if True:
                        nkb = nkb_of(b, qc)

                        def off(kb):
                            return (max(0, kb * P - qc * QC)
                                    if causal[b] else 0)

                        def scores_exp(h):
                            # pass 1: scores + exp (+ causal diag mask);
                            # returns the pt tile holding exp'd scores.
                            # One pending P3 unit per two score blocks keeps
                            # the PE fed at ScalarE's exp rate.
                            pt = ptpool.tile([P, NKB, QC], BF, tag="pt",
                                             name=f"pt{b}_{qc}_{h}")
                            for kb in range(nkb):
                                o = off(kb)
                                w = QC - o
                                sc_ps = scpool.tile([P, QC], F32)
                                nc.tensor.matmul(
                                    sc_ps[:, :w],
                                    kt_all[:, b, kb * P:(kb + 1) * P],
                                    qt_all[:, b, h,
                                           qc * QC + o:(qc + 1) * QC],
                                    start=True, stop=True)
                                nc.scalar.activation(
                                    pt[:, kb, o:QC], sc_ps[:, :w],
                                    mybir.ActivationFunctionType.Exp,
                                    scale=SCALE)
                                if causal[b] and kb * P >= qc * QC:
                                    nc.vector.tensor_mul(
                                        pt[:, kb, o:o + P],
                                        pt[:, kb, o:o + P], tri_sb[:])
                                if kb % 2 == 1:
                                    fill(1)
                            return pt

                        def sums_ot_norm(h, pt):
                            sm_ps = smpool.tile([P, QC], F32, tag="sm",
                                                name=f"sm{b}_{qc}_{h}")
                            ot_ps = oppool.tile([P, QC], F32, tag="op",
                                                name=f"op{b}_{qc}_{h}")
                            # pass 2: row sums (pre-suppression)
                            coltiled = nkb > 4 or not causal[b]
                            if not coltiled:
                                # classic full-array accumulate; sums land
                                # replicated in sm_ps.
                                for kb in range(nkb):
                                    o = off(kb)
                                    nc.tensor.matmul(
                                        sm_ps[:, o:QC], ones_sb[:],
                                        pt[:, kb, o:QC],
                                        start=(kb == 0), stop=(kb == nkb - 1))
                            else:
                                # 4-way column-tiled: strip j accumulates
                                # kb = j mod 4 into partitions 32j..32j+31.
                                # Strip j's first block is kb=j (full width,
                                # off()==0 since nkb>4 implies qc>=1), so
                                # every strip's start covers all columns.
                                for kb in range(nkb):
                                    j = kb % 4
                                    o = off(kb)
                                    nc.tensor.matmul(
                                        sm_ps[32 * j:32 * (j + 1), o:QC],
                                        on32_sb[:], pt[:, kb, o:QC],
                                        start=(kb < 4), stop=(kb >= nkb - 4),
                                        tile_position=(0, 32 * j),
                                        skip_group_check=True)

                            # pass 3: suppression column multiplies
                            sp_qc, sp_tl, sp_kbs = sup_plan[b]
                            if qc == sp_qc:
                                for kb in sp_kbs:
                                    if kb < nkb and sp_tl >= off(kb):
                                        nc.vector.tensor_mul(
                                            pt[:, kb, sp_tl:sp_tl + 1],
                                            pt[:, kb, sp_tl:sp_tl + 1],
                                            sup_sb[:, b, kb:kb + 1])

                            # pass 4: OT accumulation
                            for kb in range(nkb):
                                o = off(kb)
                                nc.tensor.matmul(
                                    ot_ps[:, o:QC], v_all[:, b, kb, :],
                                    pt[:, kb, o:QC],
                                    start=(kb == 0), stop=(kb == nkb - 1))

                            fill(1)
                            rec = recpool.tile([P, QC], F32, tag="rec")
                            if coltiled:
                                # evict strip sums to bf16 SBUF, then a
                                # single ones[128,128] matmul re-sums the 4
                                # strips (32 replicas x 4 strips x 1/32 =
                                # exact row total, replicated across all
                                # partitions) into a recycled PSUM bank.
                                sme = smepool.tile([P, QC], BF, tag="sme")
                                nc.vector.tensor_copy(sme[:], sm_ps[:])
                                sm2 = smpool.tile([P, QC], F32, tag="sm",
                                                  name=f"sm2_{b}_{qc}_{h}")
                                nc.tensor.matmul(sm2[:], ones_sb[:], sme[:],
                                                 start=True, stop=True)
                                nc.vector.reciprocal_approx_fast(
                                    rec[:], sm2[:])
                            else:
                                nc.vector.reciprocal_approx_fast(
                                    rec[:], sm_ps[:])
                            nc.vector.tensor_mul(
                                ot_all[:, b, h, qc * QC:(qc + 1) * QC],
                                ot_ps[:], rec[:])

                        # software-pipelined: scores(h) issue while
                        # sums/OT/norm(h-1) consume the previous head's
                        # exp'd scores, so the PE always has ready work
                        # during ScalarE's exp stream.
                        pts = {}
                        for h in range(HQ):
                            pts[h] = scores_exp(h)
                            if h >= 1:
                                sums_ot_norm(h - 1, pts.pop(h - 1))
                        sums_ot_norm(HQ - 1, pts.pop(HQ - 1))
                        fill(2)

                        # queue this chunk's P3 units; they drain as filler
                        # during the next chunk's score streams. Flush any
                        # excess the next chunk cannot absorb now, so no
                        # bare-PE backlog piles up at the very end.
                        for tb in range(qc * (QC // P), (qc + 1) * (QC // P)):
                            for mc in range(HID // MC):
                                pending.append((b, tb, mc))
                        if oi + 1 < len(order):
                            nb, nqc = order[oi + 1]
                            fill(len(pending) - slots_of(nb, nqc))

                fill(len(pending))
    nc.compile()
    return nc


_PROG_CACHE = {}

# Set by a test harness to capture HW profiles: TRACE=True makes kernel()
# pass trace=True to run_bass_kernel_spmd and stash the BassKernelResults
# in LAST_RESULTS. The graded path leaves these defaults alone.
TRACE = False
TRACE_KWARGS = {}
LAST_RESULTS = None


def _mask_mode(attention_mask):
    """Classify the additive mask per batch: True=causal, False=all-zero."""
    m = np.asarray(attention_mask)[:, 0]          # [B, S, S]
    modes = []
    ql, kl = np.tril_indices(S)
    qu, ku = np.triu_indices(S, k=1)
    for b in range(B):
        if np.all(m[b] == 0.0):
            modes.append(False)
        elif np.all(m[b][ql, kl] == 0.0) and np.all(m[b][qu, ku] < -1e30):
            modes.append(True)
        else:
            raise NotImplementedError(
                "attention_mask must be causal or all-zero per batch")
    return tuple(modes)


def kernel(hidden_states, Wq, Wk, Wv, Wo, attention_mask, position_ids,
           tgt_pos, subject_positions):
    hidden_states = np.asarray(hidden_states, dtype=np.float32)
    Wq = np.asarray(Wq, dtype=np.float32)
    Wk = np.asarray(Wk, dtype=np.float32)
    Wv = np.asarray(Wv, dtype=np.float32)
    Wo = np.asarray(Wo, dtype=np.float32)
    position_ids = np.asarray(position_ids)
    tgt_pos = np.asarray(tgt_pos)
    subject_positions = np.asarray(subject_positions)

    bf = ml_dtypes.bfloat16

    # ---- host-side constant prep -----------------------------------------
    # XT relayout to per-(chunk, ktile) contiguous blocks so DMA lines are
    # KS*TC*2 = 4KB: XT5[ch, kt, p, ks, t] = X^T[kt*512+ks*128+p, ch*512+t].
    XTf = np.ascontiguousarray(hidden_states.reshape(T, HID).T)   # [HID, T]
    XT5 = np.ascontiguousarray(
        XTf.reshape(NKT, KS, P, NCH, TC).transpose(3, 0, 2, 1, 4)).astype(bf)

    inv = 1.0 / (THETA ** (np.arange(0, HD, 2, dtype=np.float64) / HD))
    freqs = position_ids.astype(np.float64)[:, :, None] * inv[None, None, :]
    emb = np.concatenate([freqs, freqs], axis=-1)          # [B, S, HD]
    CS = np.ascontiguousarray(np.cos(emb).reshape(T, HD).T).astype(bf)
    SN = np.ascontiguousarray(np.sin(emb).reshape(T, HD).T).astype(bf)

    R = np.zeros((HD, HD), dtype=np.float32)               # lhsT of rotate_half
    for dout in range(HD // 2):
        R[dout + HD // 2, dout] = -1.0
    for dout in range(HD // 2, HD):
        R[dout - HD // 2, dout] = 1.0
    R = R.astype(bf)

    TRI = np.triu(np.ones((P, P), dtype=np.float32)).astype(bf)
    ONES = np.ones((P, P), dtype=bf)
    ON32 = np.full((P, 32), 1.0 / 32.0, dtype=bf)

    M = np.ones((B, S), dtype=np.float32)
    for b in range(B):
        np.multiply.at(M[b], subject_positions[b].astype(np.int64), SUPPRESS)
    SUP = np.ascontiguousarray(
        M.reshape(B, NKB, P).transpose(2, 0, 1))           # [P, B, NKB]

    sup_plan = []
    for b in range(B):
        tb = int(tgt_pos[b])
        kbs = tuple(sorted({int(v) // P for v in subject_positions[b]}))
        sup_plan.append((tb // QC, tb % QC, kbs))
    causal = _mask_mode(attention_mask)
    prog_key = (tuple(sup_plan), causal)

    if prog_key not in _PROG_CACHE:
        _PROG_CACHE[prog_key] = _build_program(sup_plan, causal)
    nc = _PROG_CACHE[prog_key]

    in_maps = []
    for i in range(NCORES):
        wq_i = Wq[i * 512:(i + 1) * 512].T                 # [HID, 512]
        wk_i = Wk[i * HD:(i + 1) * HD].T                   # [HID, 128]
        wv_i = Wv[i * HD:(i + 1) * HD].T
        wqkv_i = np.concatenate([wq_i, wk_i, wv_i], axis=1)  # [HID, 768]
        # relayout to [NKT, P, KS, WC] contiguous (6KB lines per partition)
        wqkv_i = np.ascontiguousarray(
            wqkv_i.reshape(NKT, KS, P, WC).transpose(0, 2, 1, 3)).astype(bf)
        wo_i = np.ascontiguousarray(Wo[:, i * 512:(i + 1) * 512].T).astype(bf)
        in_maps.append(dict(
            xt=XT5, wqkv=wqkv_i, wo=wo_i, cs=CS, sn=SN, rmat=R, tri=TRI,
            ones=ONES, on32=ON32, sup=SUP))

    global LAST_RESULTS
    kw = dict(TRACE_KWARGS)
    if TRACE:
        kw.setdefault("trace", True)
    res = run_bass_kernel_spmd(nc, in_maps, list(range(NCORES)), **kw)
    LAST_RESULTS = res
    total = res.results[0]["out"].astype(np.float32)
    for i in range(1, NCORES):
        total = total + res.results[i]["out"].astype(np.float32)
    return total.astype(np.float32).reshape(B, S, HID)
